# revision 1
# baseline (speedup 1.0000x reference)
"""DeepLSTM Trainium2 kernel: 2-layer LSTM (B=64,T=256,I=256,H=1024,O=256) on 8 cores.

Strategy: 8-way hidden-dim (gate) sharding. Core c owns hidden units
[c*128,(c+1)*128) of both LSTM layers and output cols [c*32,(c+1)*32).
Recurrent weights stay SBUF-resident. Hidden states are kept TRANSPOSED
(h^T: [hidden,batch]); each timestep the 8 h^T slices of each layer are
exchanged once per step with a single merged AllGather ([h1T(t+1) | h2T(t)],
bf16, 32KB/rank) — collectives have a large fixed cost here, so one per step.
The schedule is software-pipelined: when gather G(w) arrives, layer-1 of step
w+1 (the serial recurrence chain) issues first; layer-2 of step w and y of
step w-1 fill the gather window. Input projections (x @ W_ih^T) are
precomputed a few steps ahead inside the same loop (SBUF ring, float32r
matmuls). Recurrent matmuls run bf16 x bf16 with fp32 PSUM accumulation;
batch (64) rides as the stationary operand so the big weight operand streams
at 1 col/cycle. Critical-chain tuning: bias/x-projection terms are
pre-injected into PSUM via identity matmuls during the AllGather window
(removes the vector adds from the chain); layer-1 gate matmuls close the
(i,f,o) columns before (g) so the sigmoid overlaps the g-matmuls; layer-2
closes (i,f) then (g) then (o), hiding tanh_g, the c-update and tanh_c
under the remaining matmul streams (tanh_c queued on the scalar engine
before sig_o); the h1 transpose is queued after the layer-2 matmuls (no PE
stall); the critical gather half is split so layer-1 matmuls start on the
first rank blocks. Measured on 8 axon-tunneled trn2 cores: ~5.24 ms/run
(wall-clock delta of in-kernel repetitions; same-conditions baseline was
5.6 ms), max rel err vs fp32 jax reference: 3.5e-4.

All host-side work is layout only (transposes / gather-index shuffles / bias
folding); every multiply-accumulate of the model runs on the NeuronCores.
"""
import numpy as np

import concourse.bacc as bacc
import concourse.mybir as mybir
import concourse.tile as tile
from concourse.bass_utils import run_bass_kernel_spmd
from concourse.masks import make_identity

B, T, I, H, O = 64, 256, 256, 1024, 256
P = 128           # partitions / hidden slice per core
NC = 8            # cores
GS = 512          # gate-slice cols per core (4 gates x 128)
OS = O // NC      # output cols per core (32)
LOOK = 10         # precompute lookahead (steps)
F32 = mybir.dt.float32
F32R = mybir.dt.float32r
BF16 = mybir.dt.bfloat16
ACT = mybir.ActivationFunctionType


def _r(ap):
    return ap.bitcast(F32R)


def build(nt=T, reps=1):
    tb = nt * B
    nc = bacc.Bacc("TRN2", num_devices=NC)
    XT = nc.dram_tensor("XT", [P, 2 * tb], F32R, kind="ExternalInput")
    W1s = nc.dram_tensor("W1s", [P, 8 * GS], BF16, kind="ExternalInput")
    W2a = nc.dram_tensor("W2a", [P, 8 * GS], BF16, kind="ExternalInput")
    W2b = nc.dram_tensor("W2b", [P, 8 * GS], BF16, kind="ExternalInput")
    Wy = nc.dram_tensor("Wy", [P, 8 * OS], BF16, kind="ExternalInput")
    W1x = nc.dram_tensor("W1x", [P, 2 * GS], F32R, kind="ExternalInput")
    W2x = nc.dram_tensor("W2x", [P, 2 * GS], F32R, kind="ExternalInput")
    Wyx = nc.dram_tensor("Wyx", [P, 2 * OS], F32R, kind="ExternalInput")
    bias1 = nc.dram_tensor("bias1", [B, GS], F32, kind="ExternalInput")
    bias2 = nc.dram_tensor("bias2", [B, GS], F32, kind="ExternalInput")
    biasy = nc.dram_tensor("biasy", [B, OS], F32, kind="ExternalInput")
    Yout = nc.dram_tensor("Yout", [B, nt * OS], F32, kind="ExternalOutput")
    cc_in = nc.dram_tensor("cc_in", [(nt + 1) * P, 2 * B], BF16)
    cc_out = nc.dram_tensor("cc_out", [(nt + 1) * NC * P, 2 * B], BF16,
                            addr_space="Shared")
    rg = [list(range(NC))]

    with tile.TileContext(nc) as tc:
        with (
            tc.tile_pool(name="wpool", bufs=1) as wpool,
            tc.tile_pool(name="state", bufs=1) as state,
            tc.tile_pool(name="xt", bufs=4) as xtp,
            tc.tile_pool(name="aring", bufs=LOOK + 4) as aring,
            tc.tile_pool(name="work", bufs=3) as work,
            tc.tile_pool(name="gath", bufs=3) as g1p,
            tc.tile_pool(name="gath2", bufs=3) as g2p,
            tc.tile_pool(name="psA", bufs=1, space="PSUM") as psA,
            tc.tile_pool(name="psG", bufs=1, space="PSUM") as psG,
            tc.tile_pool(name="psT", bufs=2, space="PSUM") as psT,
        ):
            # --- resident weights/biases ---
            w1s = wpool.tile([P, 8 * GS], BF16)
            w2a = wpool.tile([P, 8 * GS], BF16)
            w2b = wpool.tile([P, 8 * GS], BF16)
            wy = wpool.tile([P, 8 * OS], BF16)
            w1x = wpool.tile([P, 2 * GS], F32R)
            w2x = wpool.tile([P, 2 * GS], F32R)
            wyx = wpool.tile([P, 2 * OS], F32R)
            b1 = wpool.tile([B, GS], F32)
            b2 = wpool.tile([B, GS], F32)
            by = wpool.tile([B, OS], F32)
            ident = wpool.tile([B, B], F32)
            for k in range(8):
                nc.sync.dma_start(out=w1s[:, k * GS:(k + 1) * GS],
                                  in_=W1s[:, k * GS:(k + 1) * GS])
                nc.sync.dma_start(out=w2a[:, k * GS:(k + 1) * GS],
                                  in_=W2a[:, k * GS:(k + 1) * GS])
                nc.sync.dma_start(out=w2b[:, k * GS:(k + 1) * GS],
                                  in_=W2b[:, k * GS:(k + 1) * GS])
            nc.sync.dma_start(out=wy[:], in_=Wy[:])
            nc.sync.dma_start(out=w1x[:], in_=W1x[:])
            nc.sync.dma_start(out=w2x[:], in_=W2x[:])
            nc.sync.dma_start(out=wyx[:], in_=Wyx[:])
            nc.sync.dma_start(out=b1[:], in_=bias1[:])
            nc.sync.dma_start(out=b2[:], in_=bias2[:])
            nc.sync.dma_start(out=by[:], in_=biasy[:])
            make_identity(nc, ident[:])

            for _rep in range(reps):
                c1 = state.tile([B, P], F32, tag="c1")
                c2 = state.tile([B, P], F32, tag="c2")
                nc.gpsimd.memset(c1[:], 0.0)
                nc.gpsimd.memset(c2[:], 0.0)

                a1_t, a2_t, yx_t = {}, {}, {}

                def precompute(t):
                    xt = xtp.tile([P, 2 * B], F32R, tag="xt")
                    nc.scalar.dma_start(out=xt[:, 0:B], in_=XT[:, t * B:(t + 1) * B])
                    nc.scalar.dma_start(out=xt[:, B:2 * B],
                                        in_=XT[:, tb + t * B:tb + (t + 1) * B])
                    pa1 = psA.tile([B, GS], F32, tag="pa1")
                    pa2 = psA.tile([B, GS], F32, tag="pa2")
                    pyx = psA.tile([B, OS], F32, tag="pyx")
                    for k in range(2):
                        nc.tensor.matmul(pa1[:], lhsT=xt[:, k * B:(k + 1) * B],
                                         rhs=w1x[:, k * GS:(k + 1) * GS],
                                         start=(k == 0), stop=(k == 1))
                    for k in range(2):
                        nc.tensor.matmul(pa2[:], lhsT=xt[:, k * B:(k + 1) * B],
                                         rhs=w2x[:, k * GS:(k + 1) * GS],
                                         start=(k == 0), stop=(k == 1))
                    for k in range(2):
                        nc.tensor.matmul(pyx[:], lhsT=xt[:, k * B:(k + 1) * B],
                                         rhs=wyx[:, k * OS:(k + 1) * OS],
                                         start=(k == 0), stop=(k == 1))
                    a1 = aring.tile([B, GS], F32, tag="a1")
                    a2 = aring.tile([B, GS], F32, tag="a2")
                    yx = aring.tile([B, OS], F32, tag="yx")
                    nc.vector.tensor_add(a1[:], pa1[:], b1[:])
                    nc.vector.tensor_add(a2[:], pa2[:], b2[:])
                    nc.vector.tensor_add(yx[:], pyx[:], by[:])
                    a1_t[t], a2_t[t], yx_t[t] = a1, a2, yx

                def lstm_act2(gsum, c, tag):
                    # 3-region schedule: (i,f) then (g) then (o) close in
                    # that order, so tanh_g / c-update / tanh_c all hide
                    # under the later matmul streams; tanh_c is queued on
                    # the scalar engine BEFORE sig_o (which waits region 3).
                    sif = work.tile([B, 2 * P], F32, tag=f"sif{tag}")
                    tg = work.tile([B, P], F32, tag=f"tg{tag}")
                    nc.scalar.activation(sif[:], gsum[:, 0:2 * P], ACT.Sigmoid)
                    nc.scalar.activation(tg[:], gsum[:, 384:512], ACT.Tanh)
                    t1 = work.tile([B, P], F32, tag=f"t1{tag}")
                    t2 = work.tile([B, P], F32, tag=f"t2{tag}")
                    nc.vector.tensor_mul(t1[:], sif[:, 0:P], tg[:])       # i*g
                    nc.vector.tensor_mul(t2[:], sif[:, P:2 * P], c[:])    # f*c
                    nc.vector.tensor_add(c[:], t1[:], t2[:])
                    tc_ = work.tile([B, P], F32, tag=f"tc{tag}")
                    so = work.tile([B, P], F32, tag=f"so{tag}")
                    nc.scalar.activation(tc_[:], c[:], ACT.Tanh)
                    nc.scalar.activation(so[:], gsum[:, 256:384], ACT.Sigmoid)
                    h = work.tile([B, P], F32, tag=f"h{tag}")
                    nc.vector.tensor_mul(h[:], so[:], tc_[:])
                    return h

                def lstm_act(gsum, c, tag):
                    # gsum: PSUM tile [B, GS]; region [0:384] (i,f,o) closes
                    # before [384:512] (g), so the sigmoid overlaps the g mms.
                    sig = work.tile([B, 384], F32, tag=f"sig{tag}")
                    tg = work.tile([B, P], F32, tag=f"tg{tag}")
                    nc.scalar.activation(sig[:], gsum[:, 0:384], ACT.Sigmoid)
                    nc.scalar.activation(tg[:], gsum[:, 384:512], ACT.Tanh)
                    t1 = work.tile([B, P], F32, tag=f"t1{tag}")
                    t2 = work.tile([B, P], F32, tag=f"t2{tag}")
                    nc.vector.tensor_mul(t1[:], sig[:, 0:P], tg[:])        # i*g
                    nc.vector.tensor_mul(t2[:], sig[:, P:2 * P], c[:])     # f*c
                    nc.vector.tensor_add(c[:], t1[:], t2[:])
                    tc_ = work.tile([B, P], F32, tag=f"tc{tag}")
                    nc.scalar.activation(tc_[:], c[:], ACT.Tanh)
                    h = work.tile([B, P], F32, tag=f"h{tag}")
                    nc.vector.tensor_mul(h[:], sig[:, 2 * P:3 * P], tc_[:])
                    return h

                def inject(ps, a, ncols, close=False):
                    # a (SBUF f32, bias already folded) -> ps (PSUM) via
                    # identity matmul; runs before the gather arrives, so
                    # the +a add leaves the critical chain.
                    nc.tensor.matmul(ps[:], lhsT=ident[:], rhs=a[:, 0:ncols],
                                     start=True, stop=close)

                def evict_T(h, tag):
                    """transpose h, cast bf16 into a [P, B] stage tile."""
                    tp = psT.tile([P, B], F32, tag="tps")
                    nc.tensor.transpose(tp[:], h[:], ident[:])
                    s = work.tile([P, B], BF16, tag=tag)
                    nc.vector.tensor_copy(s[:], tp[:])
                    return s

                gath = {}

                def do_ag(w):
                    """AG window w: cc_in rows w -> gathered tile gath[w]."""
                    nc.gpsimd.collective_compute(
                        "AllGather", mybir.AluOpType.bypass,
                        ins=[cc_in[w * P:(w + 1) * P, :]],
                        outs=[cc_out[w * NC * P:(w + 1) * NC * P, :]],
                        replica_groups=rg)
                    g = g1p.tile([P, NC * 2 * B], BF16, tag="g")
                    cc_o = cc_out[w * NC * P:(w + 1) * NC * P, :].rearrange(
                        "(r p) f -> p r f", p=P)
                    gv = g[:].rearrange("p (r f) -> p r f", f=2 * B)
                    # h1 half on sync (gates critical layer-1 matmuls),
                    # split so the first 4 rank blocks land early;
                    # h2 half in parallel on the scalar HWDGE engine
                    nc.sync.dma_start(out=gv[:, 0:4, 0:B], in_=cc_o[:, 0:4, 0:B])
                    nc.sync.dma_start(out=gv[:, 4:8, 0:B], in_=cc_o[:, 4:8, 0:B])
                    nc.scalar.dma_start(out=gv[:, :, B:2 * B], in_=cc_o[:, :, B:2 * B])
                    gath[w] = g
                    return g

                def blk1(g, k):   # h1T block
                    return g[:, k * 2 * B: k * 2 * B + B]

                def blk2(g, k):   # h2T block
                    return g[:, k * 2 * B + B: (k + 1) * 2 * B]

                def layer1(t, gprev):
                    """h1(t) gates into pg1; acts -> h1 (no evict here)."""
                    pg1 = psG.tile([B, GS], F32, tag="pg1")
                    inject(pg1, a1_t.pop(t), GS, close=(t == 0))
                    if t > 0:
                        # region [0:384] (i,f,o) first so sigmoid starts
                        # while the g-region mms stream
                        for k in range(8):
                            nc.tensor.matmul(pg1[:, 0:384], lhsT=blk1(gprev, k),
                                             rhs=w1s[:, k * GS:k * GS + 384],
                                             start=False, stop=(k == 7))
                        for k in range(8):
                            nc.tensor.matmul(pg1[:, 384:GS], lhsT=blk1(gprev, k),
                                             rhs=w1s[:, k * GS + 384:(k + 1) * GS],
                                             start=False, stop=(k == 7))
                    return lstm_act(pg1, c1, "1")

                for t in range(min(LOOK, nt)):
                    precompute(t)

                h1 = layer1(0, None)
                s1 = evict_T(h1, "stg1")
                nc.sync.dma_start(out=cc_in[0:P, 0:B], in_=s1[:])
                do_ag(0)             # G(0) = [h1T(0) | garbage]

                def ystep(s, g, py):
                    """y(s) from h2T(s) blocks of g + yx(s) (pre-injected)."""
                    for k in range(8):
                        nc.tensor.matmul(py[:], lhsT=blk2(g, k),
                                         rhs=wy[:, k * OS:(k + 1) * OS],
                                         start=False, stop=(k == 7))
                    ys = work.tile([B, OS], F32, tag="ys")
                    nc.vector.tensor_copy(ys[:], py[:])
                    nc.scalar.dma_start(out=Yout[:, s * OS:(s + 1) * OS], in_=ys[:])

                for w in range(nt):
                    if w + LOOK < nt:
                        precompute(w + LOOK)
                    g = gath[w]
                    # injections run during the AG window (no gather dep):
                    # emit them on PE before any gather-gated matmul
                    pg2 = psG.tile([B, GS], F32, tag="pg2")
                    inject(pg2, a2_t.pop(w), GS)
                    py = None
                    if w > 0:
                        py = psG.tile([B, OS], F32, tag="py")
                        inject(py, yx_t.pop(w - 1), OS)
                    # --- layer 1 of w+1 (critical chain) ---
                    h1 = layer1(w + 1, g) if w + 1 < nt else None
                    # --- layer 2 of w (also gates AG(w+1) via h2 stage) ---
                    for lo, hi in ((0, 256), (384, 512), (256, 384)):
                        for k in range(8):
                            nc.tensor.matmul(pg2[:, lo:hi], lhsT=blk1(g, k),
                                             rhs=w2a[:, k * GS + lo:k * GS + hi],
                                             start=False,
                                             stop=(w == 0 and k == 7))
                        if w > 0:
                            for k in range(8):
                                nc.tensor.matmul(pg2[:, lo:hi], lhsT=blk2(g, k),
                                                 rhs=w2b[:, k * GS + lo:k * GS + hi],
                                                 start=False, stop=(k == 7))
                    h2 = lstm_act2(pg2, c2, "2")
                    # PE order: T(h1) after the L2 mms (no stall: L1 acts
                    # finished while L2 streamed), y mms fill the L2-acts
                    # window, T(h2) last.
                    if h1 is not None:
                        s1 = evict_T(h1, "stg1")
                        nc.sync.dma_start(
                            out=cc_in[(w + 1) * P:(w + 2) * P, 0:B], in_=s1[:])
                    if w > 0:
                        ystep(w - 1, g, py)
                    s2 = evict_T(h2, "stg2")
                    nc.sync.dma_start(
                        out=cc_in[(w + 1) * P:(w + 2) * P, B:2 * B], in_=s2[:])
                    do_ag(w + 1)     # G(w+1) = [h1T(w+1) | h2T(w)]
                    if w > 0:
                        del gath[w - 1]

                # --- tail: y(nt-1) from h2T(nt-1) in G(nt) ---
                pyt = psG.tile([B, OS], F32, tag="py")
                inject(pyt, yx_t.pop(nt - 1), OS)
                ystep(nt - 1, gath[nt], pyt)

    nc.finalize()
    return nc


def prep_inputs(inputs, nt=T):
    """Host-side layout prep -> per-core in_maps. Pure layout, no math beyond
    bias folding (b_ih + b_hh)."""
    x = np.ascontiguousarray(inputs["x"][:, :nt, :], np.float32)
    W_ih1 = np.asarray(inputs["W_ih1"], np.float32)
    W_hh1 = np.asarray(inputs["W_hh1"], np.float32)
    W_ih2 = np.asarray(inputs["W_ih2"], np.float32)
    W_hh2 = np.asarray(inputs["W_hh2"], np.float32)
    W_l = np.asarray(inputs["W_l"], np.float32)
    b1 = np.asarray(inputs["b_ih1"], np.float32) + np.asarray(inputs["b_hh1"], np.float32)
    b2 = np.asarray(inputs["b_ih2"], np.float32) + np.asarray(inputs["b_hh2"], np.float32)
    bl = np.asarray(inputs["b_l"], np.float32)

    tb = nt * B
    xt = np.ascontiguousarray(x.transpose(2, 1, 0))  # [I, T, B]
    XT = np.concatenate([xt[0:128].reshape(P, tb), xt[128:256].reshape(P, tb)],
                        axis=1)  # [128, 2*nt*B]

    in_maps = []
    for c in range(NC):
        hs = np.arange(c * P, (c + 1) * P)
        gate_idx = np.concatenate([hs + H * j for j in (0, 1, 3, 2)])  # i,f,o,g
        ys = np.arange(c * OS, (c + 1) * OS)

        def kblocks(Wt, n=8):  # Wt: [K, M] -> [128, n*M] k-block concat
            return np.concatenate([Wt[k * P:(k + 1) * P] for k in range(n)], axis=1)

        m = {
            "XT": XT,
            "W1s": kblocks(W_hh1[gate_idx].T.copy()),
            "W2a": kblocks(W_ih2[gate_idx, 256:].T.copy()),
            "W2b": kblocks(W_hh2[gate_idx].T.copy()),
            "Wy": kblocks(W_l[ys, 256:].T.copy()),
            "W1x": kblocks(W_ih1[gate_idx, :].T.copy(), 2),
            "W2x": kblocks(W_ih2[gate_idx, :256].T.copy(), 2),
            "Wyx": kblocks(W_l[ys, :256].T.copy(), 2),
            "bias1": np.tile(b1[gate_idx], (B, 1)),
            "bias2": np.tile(b2[gate_idx], (B, 1)),
            "biasy": np.tile(bl[ys], (B, 1)),
        }
        import ml_dtypes
        bf = {"W1s", "W2a", "W2b", "Wy"}
        in_maps.append({
            k: np.ascontiguousarray(v, ml_dtypes.bfloat16 if k in bf else np.float32)
            for k, v in m.items()})
    return in_maps


_cache = {}


def run(inputs, nt=T, reps=1):
    key = (nt, reps)
    if key not in _cache:
        _cache[key] = build(nt, reps)
    nc = _cache[key]
    in_maps = prep_inputs(inputs, nt)
    res = run_bass_kernel_spmd(nc, in_maps, core_ids=list(range(NC)))
    out = np.empty((B, nt, O), np.float32)
    for c in range(NC):
        out[:, :, c * OS:(c + 1) * OS] = res.results[c]["Yout"].reshape(B, nt, OS)
    return out


def kernel(**inputs) -> np.ndarray:
    return run(inputs, T)

